# revision 6
# baseline (speedup 1.0000x reference)
"""Trainium2 Bass kernel for nn_ActualBioInspiredModel (moe_routing).

Strategy:
  - The dense path (proj -> phasor features -> 4-expert mix -> ctx) is tiny
    (~1024x84 matmuls); it is replicated on all 8 cores -> no collectives.
  - The spiking-attention scatter/top-k over the vocab reduces analytically to
    "double the argmax-|ctx[0]| column of ctx" (indices are < 64, decay weights
    are 0.7^k with a single weight >= THETA).
  - The big output projection attended @ W_out (64 x 100000) is sharded
    column-wise (vocab) across the 8 cores: each core computes a
    (1024, 12500) slab in bf16 and writes it out; the host concatenates and
    casts back to f32.
"""

import numpy as np

_B, _DIN, _HID, _E, _ED, _V = 1024, 128, 64, 4, 16, 100000
_H = 10
_DELTA0 = 7.0
_NC = 8
_VSH = _V // _NC            # 12500 vocab columns per core
_NT = 500                   # vocab tile (one PSUM bank at fp32)
_NTILES = _VSH // _NT       # 25
_DMA_GROUPS = (10, 10, 5)   # n-tiles per output DMA
_MAGIC = 12582912.0         # 1.5 * 2**23: fp32 round-to-nearest-int trick
_TWO_PI = float(2.0 * np.pi)


def _consts_array():
    ident = np.eye(128, dtype=np.float32)
    # rep4_64[e, e*16+o] = 1 : replicate gate rows into (e,o) rows
    rep4_64 = np.kron(np.eye(4, dtype=np.float32), np.ones((1, 16), np.float32))
    # rep64_16[(e,o), o'] = (o == o') : sum the 4 expert groups
    rep64_16 = np.tile(np.eye(16, dtype=np.float32), (4, 1))
    f = (_DELTA0 * np.arange(1, _H + 1, dtype=np.float32)) / (64.0 * _TWO_PI)
    fr2 = np.concatenate([f, f]).astype(np.float32)            # (20,) cos rows then sin rows
    cos_off = np.concatenate(
        [np.full(10, 0.25, np.float32), np.zeros(10, np.float32)]
    )                                                          # +0.25 turns sin into cos
    ones64 = np.ones(64, np.float32)
    ones4 = np.ones(4, np.float32)
    names = ["ident", "rep4_64", "rep64_16", "fr2", "cos_off", "ones64", "ones4"]
    parts = [ident.ravel(), rep4_64.ravel(), rep64_16.ravel(), fr2, cos_off, ones64, ones4]
    offs, cur = {}, 0
    for name, arr in zip(names, parts):
        offs[name] = cur
        cur += arr.size
    return np.ascontiguousarray(np.concatenate(parts)), offs


def _build(consts_offs, n_consts):
    import concourse.bass as bass
    import concourse.tile as tile
    from concourse import bacc, mybir

    f32 = mybir.dt.float32
    bf16 = mybir.dt.bfloat16
    Act = mybir.ActivationFunctionType
    Alu = mybir.AluOpType
    Axis = mybir.AxisListType

    nc = bacc.Bacc("TRN2", target_bir_lowering=False, debug=False)

    din = {}
    for name, shape in [
        ("x", (_B, _DIN)), ("W_in", (_DIN, _HID)), ("b_in", (_HID,)),
        ("Wg", (84, _E)), ("bg", (_E,)), ("We", (_E, 84, _ED)),
        ("be", (_E, _ED)), ("Wo", (_ED, _HID)), ("bo", (_HID,)),
        ("W_out", (_HID, _VSH)), ("b_out", (_VSH,)), ("consts", (n_consts,)),
    ]:
        din[name] = nc.dram_tensor(name, shape, f32, kind="ExternalInput").ap()
    out_ap = nc.dram_tensor("out", (_B, _VSH), bf16, kind="ExternalOutput").ap()

    CHUNKS = ((0, 512), (512, 512))  # batch free-dim chunks for dense matmuls

    with tile.TileContext(nc) as tc:
        with (
            tc.tile_pool(name="const", bufs=1) as cp,
            tc.tile_pool(name="wts", bufs=1) as wp,
            tc.tile_pool(name="dense", bufs=1) as dp,
            tc.tile_pool(name="slabs", bufs=4) as sp,
            tc.tile_pool(name="dpsum", bufs=2, space="PSUM") as dps,
            tc.tile_pool(name="mpsum", bufs=6, space="PSUM") as mps,
        ):
            def cload(name, shape):
                t = cp.tile(list(shape), f32, tag=name)
                off = consts_offs[name]
                n = int(np.prod(shape))
                src = din["consts"][off:off + n]
                if len(shape) == 2:
                    src = src.rearrange("(p q) -> p q", q=shape[1])
                nc.sync.dma_start(t[:], src)
                return t

            ident = cload("ident", (128, 128))
            rep4_64 = cload("rep4_64", (4, 64))
            rep64_16 = cload("rep64_16", (64, 16))
            fr2 = cload("fr2", (1, 20))
            cos_off = cload("cos_off", (20, 1))
            ones64 = cload("ones64", (64, 1))
            ones4r = cload("ones4", (1, 4))

            # ---- small weights into SBUF ----
            W_in_sb = wp.tile([128, 64], f32, tag="W_in")
            nc.sync.dma_start(W_in_sb[:], din["W_in"][:, :])
            Wg_a = wp.tile([64, 4], f32, tag="Wg_a")
            nc.sync.dma_start(Wg_a[:], din["Wg"][0:64, :])
            Wg_bc = wp.tile([20, 4], f32, tag="Wg_bc")
            nc.sync.dma_start(Wg_bc[:], din["Wg"][64:84, :])
            WeA = wp.tile([64, 64], f32, tag="WeA")
            WeBC = wp.tile([20, 64], f32, tag="WeBC")
            for e in range(_E):
                nc.sync.dma_start(WeA[:, e * 16:(e + 1) * 16], din["We"][e, 0:64, :])
                nc.sync.dma_start(WeBC[:, e * 16:(e + 1) * 16], din["We"][e, 64:84, :])
            Wo_sb = wp.tile([16, 64], f32, tag="Wo")
            nc.sync.dma_start(Wo_sb[:], din["Wo"][:, :])
            b_in_c = wp.tile([64, 1], f32, tag="b_in")
            nc.sync.dma_start(b_in_c[:], din["b_in"][:, None])
            bg_c = wp.tile([4, 1], f32, tag="bg")
            nc.sync.dma_start(bg_c[:], din["bg"][:, None])
            be_c = wp.tile([64, 1], f32, tag="be")
            nc.sync.dma_start(be_c[:], din["be"].rearrange("e o -> (e o)")[:, None])
            bo_c = wp.tile([64, 1], f32, tag="bo")
            nc.sync.dma_start(bo_c[:], din["bo"][:, None])

            # ---- big weight shard: f32 -> bf16 cast during DMA (SWDGE) ----
            w_sb = wp.tile([65, _VSH], bf16, tag="w_out")
            nc.gpsimd.dma_start(w_sb[0:64, :], din["W_out"][:, :])
            nc.gpsimd.dma_start(w_sb[64:65, :], din["b_out"][None, :])

            # ---- x load + transpose: xT[d, b] ----
            x_sb = dp.tile([128, _B], f32, tag="x_sb")
            nc.sync.dma_start(
                x_sb[:].rearrange("p (t d) -> p t d", d=128),
                din["x"].rearrange("(t p) d -> p t d", p=128),
            )
            xT = dp.tile([128, _B], f32, tag="xT")
            for t in range(_B // 128):
                ps = dps.tile([128, 128], f32, tag="dp")
                nc.tensor.transpose(ps[:], x_sb[:, t * 128:(t + 1) * 128], ident[:])
                nc.vector.tensor_copy(xT[:, t * 128:(t + 1) * 128], ps[:])

            # ---- proj^T = W_in.T @ xT + b_in ----
            projT = dp.tile([64, _B], f32, tag="projT")
            for c0, cn in CHUNKS:
                ps = dps.tile([64, 512], f32, tag="dp")
                nc.tensor.matmul(ps[:], W_in_sb[:], xT[:, c0:c0 + cn])
                nc.scalar.activation(projT[:, c0:c0 + cn], ps[:], Act.Identity,
                                     bias=b_in_c[:], scale=1.0)

            # ---- xmsum = column sums of proj^T (i.e. sum over HID per sample) ----
            xmsum = dp.tile([1, _B], f32, tag="xmsum")
            for c0, cn in CHUNKS:
                ps = dps.tile([1, 512], f32, tag="dp")
                nc.tensor.matmul(ps[:], ones64[:], projT[:, c0:c0 + cn])
                nc.vector.tensor_copy(xmsum[:, c0:c0 + cn], ps[:])

            # ---- u2 = fr2 (x) xmsum  (+0.25 on cos rows); cs = sin(2*pi*frac(u2)) ----
            u2 = dp.tile([20, _B], f32, tag="u2")
            for c0, cn in CHUNKS:
                ps = dps.tile([20, 512], f32, tag="dp")
                nc.tensor.matmul(ps[:], fr2[:], xmsum[:, c0:c0 + cn])
                nc.scalar.activation(u2[:, c0:c0 + cn], ps[:], Act.Identity,
                                     bias=cos_off[:], scale=1.0)
            rnd = dp.tile([20, _B], f32, tag="rnd")
            nc.vector.tensor_scalar_add(rnd[:], u2[:], _MAGIC)
            nc.vector.tensor_scalar_sub(rnd[:], rnd[:], _MAGIC)
            frac = dp.tile([20, _B], f32, tag="frac")
            nc.vector.tensor_sub(frac[:], u2[:], rnd[:])
            cs = dp.tile([20, _B], f32, tag="cs")
            nc.scalar.activation(cs[:], frac[:], Act.Sin, bias=0.0, scale=_TWO_PI)

            # ---- gate logits -> exp ----
            gate_e = dp.tile([4, _B], f32, tag="gate_e")
            for c0, cn in CHUNKS:
                ps = dps.tile([4, 512], f32, tag="dp")
                nc.tensor.matmul(ps[:], Wg_a[:], projT[:, c0:c0 + cn],
                                 start=True, stop=False)
                nc.tensor.matmul(ps[:], Wg_bc[:], cs[:, c0:c0 + cn],
                                 start=False, stop=True)
                nc.scalar.activation(gate_e[:, c0:c0 + cn], ps[:], Act.Exp,
                                     bias=bg_c[:], scale=1.0)

            # ---- softmax denominator and its reciprocal, replicated to 4 rows ----
            s_row = dp.tile([1, _B], f32, tag="s_row")
            for c0, cn in CHUNKS:
                ps = dps.tile([1, 512], f32, tag="dp")
                nc.tensor.matmul(ps[:], ones64[0:4, :], gate_e[:, c0:c0 + cn])
                nc.vector.tensor_copy(s_row[:, c0:c0 + cn], ps[:])
            r_row = dp.tile([1, _B], f32, tag="r_row")
            nc.vector.reciprocal(r_row[:], s_row[:])
            gate4 = dp.tile([4, _B], f32, tag="gate4")
            for c0, cn in CHUNKS:
                ps = dps.tile([4, 512], f32, tag="dp")
                nc.tensor.matmul(ps[:], ones4r[:], r_row[:, c0:c0 + cn])
                nc.vector.tensor_mul(gate4[:, c0:c0 + cn], gate_e[:, c0:c0 + cn], ps[:])

            # ---- experts: eo^T = tanh(We.T @ enhanced + be) ----
            eoT = dp.tile([64, _B], f32, tag="eoT")
            for c0, cn in CHUNKS:
                ps = dps.tile([64, 512], f32, tag="dp")
                nc.tensor.matmul(ps[:], WeA[:], projT[:, c0:c0 + cn],
                                 start=True, stop=False)
                nc.tensor.matmul(ps[:], WeBC[:], cs[:, c0:c0 + cn],
                                 start=False, stop=True)
                nc.scalar.activation(eoT[:, c0:c0 + cn], ps[:], Act.Tanh,
                                     bias=be_c[:], scale=1.0)

            # ---- mixed^T = sum_e gate_e * eo_e ----
            z = dp.tile([64, _B], f32, tag="z")
            for c0, cn in CHUNKS:
                ps = dps.tile([64, 512], f32, tag="dp")
                nc.tensor.matmul(ps[:], rep4_64[:], gate4[:, c0:c0 + cn])
                nc.vector.tensor_mul(z[:, c0:c0 + cn], eoT[:, c0:c0 + cn], ps[:])
            mixed = dp.tile([16, _B], f32, tag="mixed")
            for c0, cn in CHUNKS:
                ps = dps.tile([16, 512], f32, tag="dp")
                nc.tensor.matmul(ps[:], rep64_16[:], z[:, c0:c0 + cn])
                nc.vector.tensor_copy(mixed[:, c0:c0 + cn], ps[:])

            # ---- ctx^T = Wo.T @ mixed^T + bo ----
            ctxT = dp.tile([64, _B], f32, tag="ctxT")
            for c0, cn in CHUNKS:
                ps = dps.tile([64, 512], f32, tag="dp")
                nc.tensor.matmul(ps[:], Wo_sb[:], mixed[:, c0:c0 + cn])
                nc.scalar.activation(ctxT[:, c0:c0 + cn], ps[:], Act.Identity,
                                     bias=bo_c[:], scale=1.0)

            # ---- routing: gains = 1 + (|ctx[0,:]| == max) ----
            ps_row = dps.tile([1, 64], f32, tag="dp")
            nc.tensor.transpose(ps_row[:], ctxT[:, 0:1], ident[0:64, 0:64])
            abs_row = dp.tile([1, 64], f32, tag="abs_row")
            nc.scalar.activation(abs_row[:], ps_row[:], Act.Abs)
            m_sb = dp.tile([1, 1], f32, tag="m_sb")
            nc.vector.tensor_reduce(m_sb[:], abs_row[:], Axis.X, Alu.max)
            gains_row = dp.tile([1, 64], f32, tag="gains_row")
            nc.vector.tensor_scalar(gains_row[:], abs_row[:], m_sb[:], 1.0,
                                    Alu.is_equal, Alu.add)
            ps_col = dps.tile([64, 1], f32, tag="dp")
            nc.tensor.transpose(ps_col[:], gains_row[:], ident[0:1, 0:1])
            gains_c = dp.tile([64, 1], f32, tag="gains_c")
            nc.vector.tensor_copy(gains_c[:], ps_col[:])

            # ---- attended^T in bf16, with a ones row for the b_out fold ----
            attT = dp.tile([65, _B], bf16, tag="attT")
            nc.vector.tensor_scalar(attT[0:64, :], ctxT[:], gains_c[:], None,
                                    Alu.mult)
            nc.vector.memset(attT[64:65, :], 1.0)

            # ---- big GEMM: out[m*128:(m+1)*128, :] = attT_m.T @ w_sb ----
            for m in range(_B // 128):
                lhs = attT[:, m * 128:(m + 1) * 128]
                g0 = 0
                for gsz in _DMA_GROUPS:
                    slab = sp.tile([128, gsz * _NT], bf16, tag="slab")
                    for j in range(gsz):
                        n = g0 + j
                        ps = mps.tile([128, _NT], f32, tag="mm")
                        nc.tensor.matmul(ps[:], lhs,
                                         w_sb[:, n * _NT:(n + 1) * _NT])
                        dst = slab[:, j * _NT:(j + 1) * _NT]
                        if n % 2 == 0:
                            nc.vector.tensor_copy(dst, ps[:])
                        else:
                            nc.scalar.copy(dst, ps[:])
                    nc.sync.dma_start(
                        out_ap[m * 128:(m + 1) * 128,
                               g0 * _NT:(g0 + gsz) * _NT],
                        slab[:],
                    )
                    g0 += gsz

    nc.compile()
    return nc


_TRACE = False          # set by test harness to capture an NTFF profile
_LAST_RESULT = None     # BassKernelResults of the most recent run


def kernel(**inputs):
    global _LAST_RESULT
    from concourse.bass_utils import run_bass_kernel_spmd

    consts, offs = _consts_array()
    nc = _build(offs, consts.size)

    full = {k: np.ascontiguousarray(np.asarray(v, dtype=np.float32))
            for k, v in inputs.items()}
    in_maps = []
    for c in range(_NC):
        m = {k: full[k] for k in
             ("x", "W_in", "b_in", "Wg", "bg", "We", "be", "Wo", "bo")}
        m["W_out"] = np.ascontiguousarray(full["W_out"][:, c * _VSH:(c + 1) * _VSH])
        m["b_out"] = np.ascontiguousarray(full["b_out"][c * _VSH:(c + 1) * _VSH])
        m["consts"] = consts
        in_maps.append(m)

    res = run_bass_kernel_spmd(nc, in_maps, core_ids=list(range(_NC)),
                               trace=_TRACE)
    _LAST_RESULT = res
    shards = [np.asarray(res.results[c]["out"]).astype(np.float32)
              for c in range(_NC)]
    return np.concatenate(shards, axis=1)
